# revision 50
# baseline (speedup 1.0000x reference)
"""Trainium2 Bass kernel for the 3-view attention-fusion pooling module.

Computation (reference):
    t_k  = tanh(W @ x_k)                      (A=256, D=256), k = 1..3
    s_k  = h_n @ t_k                          (1, D)
    beta = softmax([s_1; s_2; s_3], axis=0)   (3, D)
    out  = beta[0]*x1 + beta[1]*x2 + beta[2]*x3   (N, D)

Sharding: rows (node dim N=100000) split evenly across 8 cores; W fed
per-core as W[:, shard].T. The (A, 3D) GEMM partials are AllReduce-summed
across cores; everything downstream of the reduction is tiny and computed
redundantly on every core.

v3 datapath (rel-err budget is 2e-2; measured ~5.5e-4 with this design):
  - host casts x1/x2/x3/W to fp16; device reads fp16 (halves HBM read)
  - GEMM in fp16 (fp32 PSUM accumulate), 1 PE cycle/row
  - batches NS.. of x stay resident in SBUF as fp16; phase 2 reads the
    stash, no reload. Batches 0..NS-1 are streamed twice: their loads and
    GEMM for iteration r+1 proceed while iteration r is still inside its
    collective+softmax bubble.
  - AllReduce payload fp16 (393 KiB), output in Shared pair-HBM
  - phase 2: out = b1*x1 + b2*x2 + b3*x3 per batch in fp16: DVE takes 4
    ops (2x fp16 mode), Pool takes 1; out stored fp16, host upcasts
  - software-pipelined emission across repeat iterations with GLOBAL tile
    pools: phase2(r) interleaves with phase1(r+1) batch-by-batch in every
    engine's program order, so in-order queues never head-of-line block
    the next iteration. Out-stores go through the Pool SWDGE queue so the
    SP/ACT HWDGE rings only carry loads.

Layout: within a batch of P*R rows, partition p holds R consecutive DRAM
rows, so every DMA moves R*D*2 contiguous bytes per partition. The GEMM
contraction is order-invariant and x / W^T / out share the row mapping,
so the permutation cancels.
"""

import sys

import numpy as np

for _p in ("/opt/trn_rl_repo", "/root/.axon_site/_ro/trn_rl_repo"):
    if _p not in sys.path:
        sys.path.append(_p)

import concourse.bacc as bacc
import concourse.tile as tile
from concourse import mybir
from concourse.bass_utils import run_bass_kernel_spmd

N_CORES = 8
N = 100000
D = 256          # feature dim
A = 256          # input_att
N_LOC = N // N_CORES   # 12500 rows per core
P = 125          # partitions per batch (matmul contraction chunk)
R = 5            # rows per partition per batch
NB = N_LOC // (P * R)  # 20 batches
FW = R * D       # free width of a batched SBUF tile (elements)
NS = 6           # batches streamed twice rather than stashed: their
                 # loads+GEMM for iteration r+1 fill the collective
                 # window of iteration r

FP32 = mybir.dt.float32
FP16 = mybir.dt.float16

Tanh = mybir.ActivationFunctionType.Tanh
Exp = mybir.ActivationFunctionType.Exp
Copy = mybir.ActivationFunctionType.Copy


class _Pools:
    pass


def _mk_pools(tc, ctx):
    p = _Pools()
    ent = ctx.enter_context
    p.pst01 = ent(tc.tile_pool(name="pst01", bufs=NB - NS))
    p.pst2 = ent(tc.tile_pool(name="pst2", bufs=NB - NS))
    p.pss01 = ent(tc.tile_pool(name="pss01", bufs=4))
    p.pss2 = ent(tc.tile_pool(name="pss2", bufs=4))
    p.pw = ent(tc.tile_pool(name="pw", bufs=2))
    p.small = ent(tc.tile_pool(name="small", bufs=1))
    p.pcc = ent(tc.tile_pool(name="pcc", bufs=2))
    p.pbeta = ent(tc.tile_pool(name="pbeta", bufs=2))
    p.pout = ent(tc.tile_pool(name="pout", bufs=3))
    p.ptmp = ent(tc.tile_pool(name="ptmp", bufs=2))
    p.pm2 = ent(tc.tile_pool(name="pm2", bufs=2))
    p.pacc = ent(tc.tile_pool(name="pacc", bufs=1, space="PSUM"))
    p.psc = ent(tc.tile_pool(name="psc", bufs=1, space="PSUM"))
    p.pdram = ent(tc.tile_pool(name="pdram", bufs=2, space="DRAM"))
    return p


def _load_batch(nc, p, xrs, b, streamed):
    """DMA the three x views of batch b. Views 1+2 land in ONE tile
    [P, 2*FW] so the GEMM can fuse them into a single 512-wide matmul.
    x1,x3 ride the SP ring; x2 the ACT ring."""
    sfx = "q" if streamed else "s"
    p01 = p.pss01 if streamed else p.pst01
    p2 = p.pss2 if streamed else p.pst2
    x12 = p01.tile([P, 2 * FW], FP16, name=f"x{sfx}01", tag=f"x{sfx}01")
    x3t = p2.tile([P, FW], FP16, name=f"x{sfx}2", tag=f"x{sfx}2")
    nc.sync.dma_start(x12[:, 0:FW], xrs[0][b])
    nc.scalar.dma_start(x12[:, FW:2 * FW], xrs[1][b])
    nc.sync.dma_start(x3t[:], xrs[2][b])
    return (x12, x3t)


def _gemm_batch(nc, uacc, wtile, xts, b, n_views=3):
    """Views 1+2 share one matmul (512-wide rhs AP over the fused x12
    tile into a full [128, 512] PSUM bank); view 3 gets its own. Each
    accumulation group owns a whole 2 KiB bank: a start=True zeroes its
    entire bank, so two open groups must never share one."""
    x12, x3t = xts
    u01, u2 = uacc
    x12v = x12[:].rearrange("p (v rd) -> p v rd", v=2)
    for g in range(R):
        first = (b == 0 and g == 0)
        last = (b == NB - 1 and g == R - 1)
        for h in range(2):
            lhs = wtile[:, g * A + h * 128: g * A + h * 128 + 128]
            nc.tensor.matmul(
                u01[h][:].rearrange("p (v d) -> p v d", v=2),
                lhsT=lhs,
                rhs=x12v[:, :, g * D:(g + 1) * D],
                start=first, stop=last)
            if n_views == 3:
                nc.tensor.matmul(
                    u2[h][:, 0:D], lhsT=lhs,
                    rhs=x3t[:, g * D:(g + 1) * D],
                    start=first, stop=last)


def _phase2_batch(nc, p, outr, Bsb, xts, b):
    """out[b] = B1*x1 + B2*x2 + B3*x3; DVE 4 ops, Pool 1 op, store on the
    Pool SWDGE queue (keeps the HWDGE load rings free of stores)."""
    x12, x3t = xts
    x1t = x12[:, 0:FW]
    x2t = x12[:, FW:2 * FW]
    m2 = p.pm2.tile([P, FW], FP16, name="m2", tag="m2")
    nc.gpsimd.tensor_mul(m2[:], x2t, Bsb[1][0:P, :])
    m1 = p.ptmp.tile([P, FW], FP16, name="m1", tag="m1")
    m3 = p.ptmp.tile([P, FW], FP16, name="m3", tag="m3")
    ot = p.pout.tile([P, FW], FP16, name="o", tag="o")
    nc.vector.tensor_mul(m1[:], x1t, Bsb[0][0:P, :])
    nc.vector.tensor_mul(m3[:], x3t[:], Bsb[2][0:P, :])
    nc.vector.tensor_add(m1[:], m1[:], m2[:])
    nc.vector.tensor_add(ot[:], m1[:], m3[:])
    nc.gpsimd.dma_start(outr[b], ot[:])


def _cc_dispatch(nc, p, uacc, n_cores, collective, shared_cc, cc_dt):
    """Partials out of PSUM, then AllReduce. Every cc-related DMA stays on
    the Pool SWDGE queue: the cc_red load waits on the collective, and an
    in-order HWDGE queue would head-of-line block the next iteration's x
    loads behind it."""
    cc_in = p.pcc.tile([128, 6 * D], cc_dt, name="cc_in", tag="cc_in")
    u01, u2 = uacc
    ccv = cc_in[:].rearrange("p (v hd) -> p v hd", v=3)
    for h in range(2):
        # u01[h] holds [v0 d | v1 d]; cc_in column for (v,h) is (2v+h)*D
        nc.scalar.activation(
            ccv[:, 0:2, h * D:(h + 1) * D],
            u01[h][:].rearrange("p (v d) -> p v d", v=2), Copy)
        if u2 is not None:
            nc.scalar.activation(cc_in[:, (4 + h) * D:(5 + h) * D],
                                 u2[h][:, 0:D], Copy)
    ccin_d = p.pdram.tile([128, 6 * D], cc_dt, name="ccin", tag="ccin")
    ccout_d = p.pdram.tile([128, 6 * D], cc_dt, name="ccout", tag="ccout",
                           addr_space="Shared" if shared_cc else "Local")
    nc.gpsimd.dma_start(ccin_d[:], cc_in[:])
    if collective:
        nc.gpsimd.collective_compute(
            "AllReduce", mybir.AluOpType.add,
            replica_groups=[list(range(n_cores))],
            ins=[ccin_d.opt()], outs=[ccout_d.opt()])
    else:
        nc.gpsimd.dma_start(ccout_d[:], ccin_d[:])
    # cc_in is dead after the store; land the reduced result in it
    nc.gpsimd.dma_start(cc_in[:], ccout_d[:])
    return cc_in


def _beta_tail(nc, p, cc_red, hn_sb, ones_sb, cc_dt):
    """tanh -> scores -> softmax -> fp16 beta broadcast tiles [128, FW]
    (double-buffered across iterations). Emitted AFTER the next
    iteration's streamed GEMM so the PE queue is not head-of-line blocked
    on the collective."""
    t_tanh = p.small.tile([128, 6 * D], FP16, name="t_tanh", tag="t_tanh")
    nc.scalar.activation(t_tanh[:], cc_red[:], Tanh)

    evs = []
    Bsb = []
    # one full-bank scores tile, reused sequentially per view (the WAR
    # against the previous view's exp read orders the groups)
    s_ps = p.psc.tile([1, 512], FP32, name="s", tag="s")
    for v in range(3):
        sv = s_ps[:, 0:D]
        for h in range(2):
            i = v * 2 + h
            nc.tensor.matmul(
                sv, lhsT=hn_sb[:, h:h + 1],
                rhs=t_tanh[:, i * D:(i + 1) * D],
                start=(h == 0), stop=(h == 1))
        e_v = p.small.tile([1, D], FP32, name=f"e{v}", tag=f"e{v}")
        nc.scalar.activation(e_v[:], sv, Exp)
        evs.append(e_v)
    ssum = p.small.tile([1, D], FP32, name="ssum", tag="ssum")
    nc.vector.tensor_add(ssum[:], evs[0][:], evs[1][:])
    nc.vector.tensor_add(ssum[:], ssum[:], evs[2][:])
    rinv = p.small.tile([1, D], FP32, name="rinv", tag="rinv")
    nc.vector.reciprocal(rinv[:], ssum[:])
    B_ps = p.psc.tile([128, 512], FP32, name="Bps", tag="Bps")
    for v in range(3):
        b_v = p.small.tile([1, D], FP16, name=f"bt{v}", tag=f"bt{v}")
        nc.vector.tensor_mul(b_v[:], evs[v][:], rinv[:])
        nc.tensor.matmul(B_ps[:, 0:D], lhsT=ones_sb[:], rhs=b_v[:],
                         start=True, stop=True)
        B_v = p.pbeta.tile([128, FW], FP16, name=f"Bb{v}", tag=f"Bb{v}")
        # R-fold tile of beta along the free dim happens in this copy
        nc.scalar.activation(
            B_v[:].rearrange("p (r d) -> p r d", r=R),
            B_ps[:, 0:D].unsqueeze(1).broadcast_to([128, R, D]), Copy)
        Bsb.append(B_v)
    return Bsb


def build_bass(n_cores=N_CORES, collective=True, repeat=1, phase2=True,
               shared_cc=True, cc_dt=FP16, n_views=3):
    nc = bacc.Bacc("TRN2", target_bir_lowering=False, debug=False,
                   num_devices=n_cores)

    x1 = nc.dram_tensor("x1", [N_LOC, D], FP16, kind="ExternalInput")
    x2 = nc.dram_tensor("x2", [N_LOC, D], FP16, kind="ExternalInput")
    x3 = nc.dram_tensor("x3", [N_LOC, D], FP16, kind="ExternalInput")
    wt = nc.dram_tensor("wt", [N_LOC, A], FP16, kind="ExternalInput")
    hnt = nc.dram_tensor("hnt", [A, 1], FP32, kind="ExternalInput")
    out = nc.dram_tensor("out", [N_LOC, D], FP16, kind="ExternalOutput")

    from contextlib import ExitStack

    with tile.TileContext(nc) as tc, ExitStack() as ctx:
        p = _mk_pools(tc, ctx)
        x1r = x1.ap().rearrange("(b p r) d -> b p (r d)", p=P, r=R)
        x2r = x2.ap().rearrange("(b p r) d -> b p (r d)", p=P, r=R)
        x3r = x3.ap().rearrange("(b p r) d -> b p (r d)", p=P, r=R)
        wtr = wt.ap().rearrange("(b p r) a -> b p (r a)", p=P, r=R)
        outr = out.ap().rearrange("(b p r) d -> b p (r d)", p=P, r=R)
        xrs = (x1r, x2r, x3r)

        # h_n laid out [a_half(128 partitions), h(2)], cast to fp16 to
        # match the fp16 tanh tile in the scores matmul
        hn32 = p.small.tile([128, 2], FP32, name="hn32", tag="hn32")
        nc.sync.dma_start(hn32[:, :],
                          hnt.ap().rearrange("(h a) o -> a (h o)", h=2))
        hn_sb = p.small.tile([128, 2], FP16, name="hn", tag="hn")
        nc.vector.tensor_copy(hn_sb[:], hn32[:])
        ones_sb = p.small.tile([1, 128], FP16, name="ones", tag="ones")
        nc.vector.memset(ones_sb[:], 1.0)

        if n_views:
            u01 = [p.pacc.tile([128, 512], FP32, name=f"u01{h}",
                               tag=f"u01{h}") for h in range(2)]
            u2 = ([p.pacc.tile([128, 512], FP32, name=f"u2{h}",
                               tag=f"u2{h}") for h in range(2)]
                  if n_views == 3 else None)
            uacc = (u01, u2)
        else:
            uacc = None

        cc_red = None   # un-reduced-yet cc result tile of iteration r-1
        pstash = None   # stash dict of iteration r-1
        for r in range(repeat):
            # streamed batches of r: their loads + GEMM fill the cc(r-1)
            # window (nothing here waits on the collective)
            for b in range(NS):
                wtile = p.pw.tile([P, R * A], FP16, name="w", tag="w")
                nc.scalar.dma_start(wtile[:], wtr[b])
                xts = _load_batch(nc, p, xrs, b, streamed=True)
                if n_views:
                    _gemm_batch(nc, uacc, wtile, xts, b, n_views)
            # beta tail of r-1 (first point that waits on cc(r-1))
            if cc_red is not None:
                pBsb = _beta_tail(nc, p, cc_red, hn_sb, ones_sb, cc_dt)
            # stash batches of r, interleaved with phase 2 of r-1
            stash = {}
            for b in range(NS, NB):
                if cc_red is not None and phase2:
                    _phase2_batch(nc, p, outr, pBsb, pstash[b], b)
                wtile = p.pw.tile([P, R * A], FP16, name="w", tag="w")
                nc.scalar.dma_start(wtile[:], wtr[b])
                xts = _load_batch(nc, p, xrs, b, streamed=False)
                stash[b] = xts
                if n_views:
                    _gemm_batch(nc, uacc, wtile, xts, b, n_views)
            # streamed batches of r-1: reload + phase 2
            if cc_red is not None and phase2:
                for b in range(NS):
                    xts = _load_batch(nc, p, xrs, b, streamed=True)
                    _phase2_batch(nc, p, outr, pBsb, xts, b)
            if n_views:
                cc_red = _cc_dispatch(nc, p, uacc, n_cores, collective,
                                      shared_cc, cc_dt)
            pstash = stash

        # drain: last iteration's beta + phase 2
        if n_views == 0:
            tok = p.pout.tile([P, FW], FP16, name="tok", tag="tok")
            nc.vector.tensor_copy(tok[:], pstash[NS][0][:, 0:FW])
            nc.gpsimd.dma_start(outr[0], tok[:])
            pBsb = None
        else:
            pBsb = _beta_tail(nc, p, cc_red, hn_sb, ones_sb, cc_dt)
        if n_views and phase2:
            for b in range(NS, NB):
                _phase2_batch(nc, p, outr, pBsb, pstash[b], b)
            for b in range(NS):
                xts = _load_batch(nc, p, xrs, b, streamed=True)
                _phase2_batch(nc, p, outr, pBsb, xts, b)
        elif n_views:
            tok = p.pout.tile([P, FW], FP16, name="tok", tag="tok")
            nc.vector.tensor_mul(tok[:], pBsb[0][0:P, :],
                                 pBsb[1][0:P, :])
            nc.gpsimd.dma_start(outr[0], tok[:])

    nc.compile()
    return nc


_NC_CACHE = {}


def _get_nc():
    if "nc" not in _NC_CACHE:
        _NC_CACHE["nc"] = build_bass()
    return _NC_CACHE["nc"]


def kernel(x1, x2, x3, W, h_n):
    x1h = np.ascontiguousarray(x1, dtype=np.float16)
    x2h = np.ascontiguousarray(x2, dtype=np.float16)
    x3h = np.ascontiguousarray(x3, dtype=np.float16)
    Wh = np.ascontiguousarray(W, dtype=np.float16)
    h_n = np.ascontiguousarray(h_n, dtype=np.float32)

    hnt = np.ascontiguousarray(h_n.reshape(-1)[:, None])  # (A, 1)
    in_maps = []
    for c in range(N_CORES):
        sl = slice(c * N_LOC, (c + 1) * N_LOC)
        in_maps.append({
            "x1": x1h[sl],
            "x2": x2h[sl],
            "x3": x3h[sl],
            "wt": np.ascontiguousarray(Wh[:, sl].T),
            "hnt": hnt,
        })

    nc = _get_nc()
    res = run_bass_kernel_spmd(nc, in_maps, core_ids=list(range(N_CORES)))
    out16 = np.concatenate([res.results[c]["out"] for c in range(N_CORES)],
                           axis=0)
    return out16.astype(np.float32)
